# revision 8
# baseline (speedup 1.0000x reference)
"""Multi-head attention (B=4, S=2048, D=1024, H=16) on 8 trn2 NeuronCores.

Sharding: tensor-parallel over heads - 2 heads per core. Each core computes
qkv for its 128 channels (column-parallel), full attention for its 8
(batch, head) pairs, and a row-parallel slice of the output projection.
Host gathers the 8 partial projection outputs and sums them (+ b_proj).

All matmul operands are bf16 (PSUM accumulation stays f32). HW-measured
rates motivating the layout (per matmul instruction, warm):
  - [128,128] x [128,N] runs at ~N*0.42 ns (full rate) for N>=256
  - contraction over only 64 partitions runs at HALF rate regardless of N
So scores use a ZERO-PADDED kT: for each 128-ki tile and head h, a
[128,128] stationary block holding kT in partition rows 64h..64h+63 and
zeros in the other 64 rows. The moving operand is the plain stacked qT
(head0 dims in partitions 0-63, head1 in 64-127), so the zero rows kill
the other head's contribution and each score matmul contracts over the
full 128 partitions at full rate. No partition shifts are needed to build
the padded kT: head h's qkv bias-add writes its rows in place.

Phase C is ScalarE-bound (softmax exp); to keep the PE busy during exp
waits, the attention loop for batch b is interleaved with PE filler work:
the output projection of batch b-1 (one 128-token block per (head,qblock)
iteration) and the qkv projections of batch b+1 (one 512-token block per
four iterations). attn@V matmuls issue group-by-group right after each
exp so po accumulation overlaps scoring of the next group. v tiles carry
a ones column so the attn@V chain also emits the softmax denominator row;
normalization is a DVE reciprocal + GpSimd partition-broadcast + one
fused multiply.
"""

import numpy as np
import ml_dtypes

import concourse.bass as bass
import concourse.mybir as mybir
import concourse.tile as tile
from concourse import bacc
from concourse.bass_utils import run_bass_kernel_spmd
from concourse.masks import make_identity

F32 = mybir.dt.float32
BF16 = mybir.dt.bfloat16

N_CORES = 8


def build_core_program(B=4, S=2048, D=1024, H=16, QB=256, TB=512, reps=1,
                       bufs_x=2, bufs_pT=3, bufs_s=2, bufs_o=2, bufs_wy=2,
                       KG=4, act_copy_frac=0):
    """One core's program (SPMD: every core runs this on its own shard).

    act_copy_frac: every act_copy_frac-th projection psum->sbuf copy goes
    to ScalarE instead of DVE (0 = all on DVE).
    """
    HD = D // H                 # 64
    HPC = H // N_CORES          # heads per core = 2
    CPC = HPC * HD              # channels per core = 128
    T = B * S                   # tokens = 8192
    SCALE = float(HD) ** -0.5
    KT = 128                    # ki tile
    NKT = S // KT               # ki tiles per batch = 16
    NTT = T // KT               # token tiles total = 64
    VW = HD + 1                 # v tile width per head incl. ones col = 65
    NQB = S // QB               # q blocks per batch = 8
    KD = D // 128               # contraction tiles for qkv = 8
    KH = KD // 2
    NG = NKT // KG              # score groups per q block = 4
    TPB = TB // 128             # 128-token tiles per qkv block = 4
    TBB = S // TB               # qkv token blocks per batch = 4
    NIT = HPC * NQB             # attention iterations per batch = 16

    nc = bacc.Bacc("TRN2", target_bir_lowering=False, debug=False,
                   num_devices=N_CORES)

    xT_d = nc.dram_tensor("xT", [D, T], BF16, kind="ExternalInput")
    wq_d = nc.dram_tensor("wq", [D, CPC], BF16, kind="ExternalInput")
    wk_d = nc.dram_tensor("wk", [D, CPC], BF16, kind="ExternalInput")
    wv_d = nc.dram_tensor("wv", [D, CPC], BF16, kind="ExternalInput")
    bq_d = nc.dram_tensor("bq", [CPC, 1], F32, kind="ExternalInput")
    bk_d = nc.dram_tensor("bk", [CPC, 1], F32, kind="ExternalInput")
    bv_d = nc.dram_tensor("bv", [CPC, 1], F32, kind="ExternalInput")
    wp_d = nc.dram_tensor("wp", [CPC, D], BF16, kind="ExternalInput")
    y_d = nc.dram_tensor("y", [T, D], F32, kind="ExternalOutput")

    with tile.TileContext(nc) as tc:
        with tc.tile_pool(name="const", bufs=1) as const, \
             tc.tile_pool(name="persist", bufs=1) as persist, \
             tc.tile_pool(name="xin", bufs=bufs_x) as xin, \
             tc.tile_pool(name="vtmp", bufs=2) as vtmp, \
             tc.tile_pool(name="pT", bufs=bufs_pT) as p_pool, \
             tc.tile_pool(name="ao", bufs=2) as ao_pool, \
             tc.tile_pool(name="rcp", bufs=3) as rcp_pool, \
             tc.tile_pool(name="rcb", bufs=3) as rcb_pool, \
             tc.tile_pool(name="yout", bufs=3) as yout, \
             tc.tile_pool(name="s_ps", bufs=bufs_s, space="PSUM") as s_ps, \
             tc.tile_pool(name="o_ps", bufs=bufs_o, space="PSUM") as o_ps, \
             tc.tile_pool(name="wy_ps", bufs=bufs_wy, space="PSUM") as wy_ps:
            xT_r = xT_d.ap().rearrange("(a p) t -> p a t", p=128)
            qT_s = persist.tile([128, T], BF16)       # stacked head dims
            kTp_s = persist.tile([128, 2 * T], BF16)  # zero-padded per head
            v_s = persist.tile([128, NTT * HPC * VW], BF16)
            # padded-kT view: [128, tile, head, 128]
            kTp_v = kTp_s[:].rearrange("p (t h c) -> p t h c", h=HPC, c=KT)

            ident = const.tile([128, 128], BF16)
            ones_bf = const.tile([128, 2], BF16)
            wq_s = const.tile([128, KD, 128], BF16)
            wk_s = const.tile([128, KD, 128], BF16)
            wv_s = const.tile([128, KD, 128], BF16)
            wp_s = const.tile([128, D], BF16)
            bq_s = const.tile([CPC, 1], F32)
            bk_s = const.tile([CPC, 1], F32)
            bv_s = const.tile([CPC, 1], F32)

            def setup():
                for w_d, w_s in ((wq_d, wq_s), (wk_d, wk_s), (wv_d, wv_s)):
                    nc.sync.dma_start(
                        w_s[:], w_d.ap().rearrange("(a p) m -> p a m", p=128))
                nc.sync.dma_start(wp_s[:], wp_d.ap()[:, :])
                nc.sync.dma_start(bq_s[:], bq_d.ap()[:, :])
                nc.sync.dma_start(bk_s[:], bk_d.ap()[:, :])
                nc.sync.dma_start(bv_s[:], bv_d.ap()[:, :])
                make_identity(nc, ident[:])
                nc.vector.memset(kTp_s[:], 0.0)
                nc.vector.memset(ones_bf[:], 1.0)
                ones_cols = v_s[:].rearrange(
                    "p (t h w) -> p t h w", t=NTT, h=HPC)[:, :, :, HD:VW]
                nc.vector.tensor_copy(
                    ones_cols,
                    ones_bf[:, 0:1].to_broadcast([128, NTT, HPC, 1]))

            def load_x(rep, tb, split=False):
                xt = xin.tile([128, KD, TB], BF16, name=f"xt_{rep}_{tb}",
                              tag="xt")
                if split:
                    nc.sync.dma_start(xt[:, 0:KH, :],
                                      xT_r[:, 0:KH, tb * TB:(tb + 1) * TB])
                    nc.sync.dma_start(xt[:, KH:KD, :],
                                      xT_r[:, KH:KD, tb * TB:(tb + 1) * TB])
                else:
                    nc.sync.dma_start(xt[:], xT_r[:, :, tb * TB:(tb + 1) * TB])
                return xt

            def emit_qkv_tb(rep, tb, xt, parts=("q", "k", "v")):
                """qkv projections + padded-kT / transposed-v writes for one
                512-token block."""
                lsl = slice(tb * TB, (tb + 1) * TB)
                sel = (("q", wq_s, bq_s), ("k", wk_s, bk_s),
                       ("v", wv_s, bv_s))
                for which, w_s, b_s in (t for t in sel if t[0] in parts):
                    ps = wy_ps.tile([128, TB], F32,
                                    name=f"ps_{which}{rep}_{tb}", tag="wy")
                    for kd in range(KD):
                        nc.tensor.matmul(ps[:], w_s[:, kd, :], xt[:, kd, :],
                                         start=(kd == 0), stop=(kd == KD - 1))
                    if which == "q":
                        nc.vector.tensor_scalar_add(qT_s[:, lsl], ps[:],
                                                    b_s[:])
                    elif which == "k":
                        ps_r = ps[:].rearrange("p (t c) -> p t c", c=KT)
                        for h in range(HPC):
                            hs = slice(h * HD, (h + 1) * HD)
                            dst = kTp_v[hs, tb * TPB:(tb + 1) * TPB, h, :]
                            nc.vector.tensor_scalar_add(
                                dst, ps_r[hs, :, :], b_s[hs, :])
                    else:
                        vt = vtmp.tile([128, TB], BF16, name=f"vt{rep}_{tb}",
                                       tag="vt")
                        nc.vector.tensor_scalar_add(vt[:], ps[:], b_s[:])
                        for j in range(TPB):
                            ti = tb * TPB + j
                            pt = wy_ps.tile([128, 128], BF16,
                                            name=f"pt{rep}_{ti}", tag="wy")
                            nc.tensor.transpose(
                                pt[:], vt[:, j * 128:(j + 1) * 128], ident[:])
                            base = ti * HPC * VW
                            dst = v_s[:, base:base + HPC * VW].rearrange(
                                "p (h w) -> p h w", h=HPC)[:, :, 0:HD]
                            src = pt[:].rearrange("p (h w) -> p h w", h=HPC)
                            nc.vector.tensor_copy(dst, src)

            def emit_proj_tt(rep, b, ao, tt):
                """output projection for one 128-token block of batch b."""
                lt = ao[:, tt * 128:(tt + 1) * 128]
                yt = yout.tile([128, D], F32, name=f"yt{rep}_{b}_{tt}",
                               tag="yt")
                for half in range(2):
                    py = wy_ps.tile([128, 512], F32,
                                    name=f"py{rep}_{b}_{tt}_{half}", tag="wy")
                    nc.tensor.matmul(py[:], lt,
                                     wp_s[:, half * 512:(half + 1) * 512],
                                     start=True, stop=True)
                    dst = yt[:, half * 512:(half + 1) * 512]
                    if act_copy_frac and (tt * 2 + half) % act_copy_frac == 0:
                        nc.scalar.activation(
                            dst, py[:], mybir.ActivationFunctionType.Copy)
                    else:
                        nc.vector.tensor_copy(dst, py[:])
                nc.sync.dma_start(
                    y_d.ap()[b * S + tt * 128: b * S + (tt + 1) * 128, :],
                    yt[:])

            def emit_attn_iter(rep, b, h, qb, ao):
                """scores + exp + attn@V + normalize for one (head, qblock)."""
                hs = slice(h * HD, (h + 1) * HD)
                qsl = slice(b * S + qb * QB, b * S + (qb + 1) * QB)
                po = o_ps.tile([VW, QB], F32, name=f"po{rep}_{b}_{h}_{qb}",
                               tag="po")
                for g in range(NG):
                    ss = s_ps.tile([128, KG, QB], F32,
                                   name=f"ss{rep}_{b}_{h}_{qb}_{g}", tag="ss")
                    for j in range(KG):
                        kt = g * KG + j
                        ti = b * NKT + kt
                        nc.tensor.matmul(ss[:, j, :], kTp_v[:, ti, h, :],
                                         qT_s[:, qsl],
                                         start=(j % 2 == 0),
                                         stop=(j % 2 == 1 or j == KG - 1))
                    pTg = p_pool.tile([128, KG, QB], BF16,
                                      name=f"pT{rep}_{b}_{h}_{qb}_{g}",
                                      tag="pT")
                    nc.scalar.activation(pTg[:], ss[:],
                                         mybir.ActivationFunctionType.Exp,
                                         scale=SCALE)
                    for j in range(KG):
                        kt = g * KG + j
                        vb = (b * NKT + kt) * HPC * VW + h * VW
                        nc.tensor.matmul(po[:], v_s[:, vb:vb + VW],
                                         pTg[:, j, :],
                                         start=(kt == 0),
                                         stop=(kt == NKT - 1))
                rc = rcp_pool.tile([1, QB], F32, name=f"rc{rep}_{b}_{h}_{qb}",
                                   tag="rc")
                nc.vector.reciprocal(rc[:], po[HD:VW, :])
                rcb = rcb_pool.tile([HD, QB], F32,
                                    name=f"rcb{rep}_{b}_{h}_{qb}", tag="rcb")
                nc.gpsimd.partition_broadcast(rcb[:], rc[:])
                nc.vector.scalar_tensor_tensor(
                    ao[hs, qb * QB:(qb + 1) * QB], po[0:HD, :], 1.0, rcb[:],
                    op0=mybir.AluOpType.mult, op1=mybir.AluOpType.mult)

            ao_tiles = {}
            for rep in range(reps):
                # phase B for batch 0 (standalone; nothing to overlap with)
                xt = load_x(rep, 0, split=True)
                if rep == 0:
                    setup()
                for tb in range(TBB):
                    xt_next = load_x(rep, tb + 1) if tb + 1 < TBB else None
                    emit_qkv_tb(rep, tb, xt)
                    xt = xt_next

                for b in range(B):
                    ao = ao_pool.tile([128, S], BF16, name=f"ao{rep}_{b}",
                                      tag="ao")
                    ao_tiles[(rep, b)] = ao
                    # prefetch first x block of batch b+1
                    if b + 1 < B:
                        xt = load_x(rep, (b + 1) * TBB)
                    slots = NIT // TBB      # iterations per qkv filler block
                    NPT = (S // 128) // NIT  # proj blocks per iteration
                    for it in range(NIT):
                        h, qb = divmod(it, NQB)
                        emit_attn_iter(rep, b, h, qb, ao)
                        # filler: proj block(s) of batch b-1 per iteration
                        if b > 0:
                            for tt in range(it * NPT, (it + 1) * NPT):
                                emit_proj_tt(rep, b - 1,
                                             ao_tiles[(rep, b - 1)], tt)
                        # filler: one qkv block of batch b+1 per `slots` iters
                        if b + 1 < B and it % slots == slots - 1:
                            tb = (b + 1) * TBB + it // slots
                            xt_next = (load_x(rep, tb + 1)
                                       if it // slots + 1 < TBB else None)
                            emit_qkv_tb(rep, tb, xt)
                            xt = xt_next
                # tail: projection of the last batch
                for tt in range(S // 128):
                    emit_proj_tt(rep, B - 1, ao_tiles[(rep, B - 1)], tt)

    nc.compile()
    return nc


def shard_inputs(x, w_qkv, b_qkv, w_proj, B=4, S=2048, D=1024, H=16):
    """Host-side sharding: returns in_maps for the 8 cores."""
    HD = D // H
    HPC = H // N_CORES
    CPC = HPC * HD
    T = B * S
    x = np.asarray(x, dtype=np.float32)
    w_qkv = np.asarray(w_qkv, dtype=np.float32)
    b_qkv = np.asarray(b_qkv, dtype=np.float32)
    w_proj = np.asarray(w_proj, dtype=np.float32)
    bf = ml_dtypes.bfloat16
    xT = np.ascontiguousarray(x.reshape(T, D).T.astype(bf))
    in_maps = []
    for c in range(N_CORES):
        sl = slice(c * CPC, (c + 1) * CPC)
        in_maps.append({
            "xT": xT,
            "wq": np.ascontiguousarray(w_qkv[:, 0 * D:1 * D][:, sl]
                                       .astype(bf)),
            "wk": np.ascontiguousarray(w_qkv[:, 1 * D:2 * D][:, sl]
                                       .astype(bf)),
            "wv": np.ascontiguousarray(w_qkv[:, 2 * D:3 * D][:, sl]
                                       .astype(bf)),
            "bq": np.ascontiguousarray(b_qkv[0 * D:1 * D][sl]).reshape(CPC, 1),
            "bk": np.ascontiguousarray(b_qkv[1 * D:2 * D][sl]).reshape(CPC, 1),
            "bv": np.ascontiguousarray(b_qkv[2 * D:3 * D][sl]).reshape(CPC, 1),
            "wp": np.ascontiguousarray(w_proj[sl, :].astype(bf)),
        })
    return in_maps


_NC_CACHE = {}


def _get_nc():
    if "nc" not in _NC_CACHE:
        _NC_CACHE["nc"] = build_core_program()
    return _NC_CACHE["nc"]


def kernel(x, w_qkv, b_qkv, w_proj, b_proj, _trace=False):
    B, S, D = 4, 2048, 1024
    nc = _get_nc()
    in_maps = shard_inputs(x, w_qkv, b_qkv, w_proj, B=B, S=S, D=D)
    res = run_bass_kernel_spmd(nc, in_maps, core_ids=list(range(N_CORES)),
                               trace=_trace)
    y = res.results[0]["y"].astype(np.float64)
    for i in range(1, N_CORES):
        y += res.results[i]["y"]
    y += np.asarray(b_proj, dtype=np.float64)
    out = y.astype(np.float32).reshape(B, S, D)
    if _trace:
        return out, res
    return out


# revision 9
# speedup vs baseline: 1.0951x; 1.0951x over previous
"""Multi-head attention (B=4, S=2048, D=1024, H=16) on 8 trn2 NeuronCores.

Sharding: tensor-parallel over heads - 2 heads per core. Each core computes
qkv for its 128 channels (column-parallel), full attention for its 8
(batch, head) pairs, and a row-parallel slice of the output projection.
Host gathers the 8 partial projection outputs and sums them (+ b_proj).

All matmul operands are bf16 (PSUM accumulation stays f32). HW-measured
rates motivating the layout (per matmul instruction, warm):
  - [128,128] x [128,N] runs at ~N*0.42 ns (full rate) for N>=256
  - contraction over only 64 partitions runs at HALF rate regardless of N
So scores use a ZERO-PADDED kT: for each 128-ki tile and head h, a
[128,128] stationary block holding kT in partition rows 64h..64h+63 and
zeros in the other 64 rows. The moving operand is the plain stacked qT
(head0 dims in partitions 0-63, head1 in 64-127), so the zero rows kill
the other head's contribution and each score matmul contracts over the
full 128 partitions at full rate. No partition shifts are needed to build
the padded kT: head h's qkv bias-add writes its rows in place.

Phase C is ScalarE-bound (softmax exp); to keep the PE busy during exp
waits, the attention loop for batch b is interleaved with PE filler work:
the output projection of batch b-1 (one 128-token block per (head,qblock)
iteration) and the qkv projections of batch b+1 (one 512-token block per
four iterations). attn@V matmuls issue group-by-group right after each
exp so po accumulation overlaps scoring of the next group. v tiles carry
a ones column so the attn@V chain also emits the softmax denominator row;
normalization is a DVE reciprocal + GpSimd partition-broadcast + one
fused multiply.
"""

import numpy as np
import ml_dtypes

import concourse.bass as bass
import concourse.mybir as mybir
import concourse.tile as tile
from concourse import bacc
from concourse.bass_utils import run_bass_kernel_spmd
from concourse.masks import make_identity

F32 = mybir.dt.float32
BF16 = mybir.dt.bfloat16

N_CORES = 8


def build_core_program(B=4, S=2048, D=1024, H=16, QB=256, TB=512, reps=1,
                       bufs_x=2, bufs_pT=3, bufs_s=2, bufs_o=2, bufs_wy=2,
                       KG=4, act_copy_frac=0):
    """One core's program (SPMD: every core runs this on its own shard).

    act_copy_frac: every act_copy_frac-th projection psum->sbuf copy goes
    to ScalarE instead of DVE (0 = all on DVE).
    """
    HD = D // H                 # 64
    HPC = H // N_CORES          # heads per core = 2
    CPC = HPC * HD              # channels per core = 128
    T = B * S                   # tokens = 8192
    SCALE = float(HD) ** -0.5
    KT = 128                    # ki tile
    NKT = S // KT               # ki tiles per batch = 16
    NTT = T // KT               # token tiles total = 64
    VW = HD + 1                 # v tile width per head incl. ones col = 65
    NQB = S // QB               # q blocks per batch = 8
    KD = D // 128               # contraction tiles for qkv = 8
    KH = KD // 2
    NG = NKT // KG              # score groups per q block = 4
    TPB = TB // 128             # 128-token tiles per qkv block = 4
    TBB = S // TB               # qkv token blocks per batch = 4
    NIT = HPC * NQB             # attention iterations per batch = 16

    nc = bacc.Bacc("TRN2", target_bir_lowering=False, debug=False,
                   num_devices=N_CORES)

    xT_d = nc.dram_tensor("xT", [D, T], BF16, kind="ExternalInput")
    wq_d = nc.dram_tensor("wq", [D, CPC], BF16, kind="ExternalInput")
    wk_d = nc.dram_tensor("wk", [D, CPC], BF16, kind="ExternalInput")
    wv_d = nc.dram_tensor("wv", [D, CPC], BF16, kind="ExternalInput")
    bq_d = nc.dram_tensor("bq", [CPC, 1], F32, kind="ExternalInput")
    bk_d = nc.dram_tensor("bk", [CPC, 1], F32, kind="ExternalInput")
    bv_d = nc.dram_tensor("bv", [CPC, 1], F32, kind="ExternalInput")
    wp_d = nc.dram_tensor("wp", [CPC, D], BF16, kind="ExternalInput")
    y_d = nc.dram_tensor("y", [T, D], F32, kind="ExternalOutput")

    with tile.TileContext(nc) as tc:
        with tc.tile_pool(name="const", bufs=1) as const, \
             tc.tile_pool(name="persist", bufs=1) as persist, \
             tc.tile_pool(name="xin", bufs=bufs_x) as xin, \
             tc.tile_pool(name="vtmp", bufs=2) as vtmp, \
             tc.tile_pool(name="pT", bufs=bufs_pT) as p_pool, \
             tc.tile_pool(name="ao", bufs=2) as ao_pool, \
             tc.tile_pool(name="rcp", bufs=3) as rcp_pool, \
             tc.tile_pool(name="rcb", bufs=3) as rcb_pool, \
             tc.tile_pool(name="yout", bufs=3) as yout, \
             tc.tile_pool(name="s_ps", bufs=bufs_s, space="PSUM") as s_ps, \
             tc.tile_pool(name="o_ps", bufs=bufs_o, space="PSUM") as o_ps, \
             tc.tile_pool(name="wy_ps", bufs=bufs_wy, space="PSUM") as wy_ps:
            xT_r = xT_d.ap().rearrange("(a p) t -> p a t", p=128)
            qT_s = persist.tile([128, T], BF16)       # stacked head dims
            kTp_s = persist.tile([128, 2 * T], BF16)  # zero-padded per head
            v_s = persist.tile([128, NTT * HPC * VW], BF16)
            # padded-kT view: [128, tile, head, 128]
            kTp_v = kTp_s[:].rearrange("p (t h c) -> p t h c", h=HPC, c=KT)

            ident = const.tile([128, 128], BF16)
            ones_bf = const.tile([128, 2], BF16)
            wq_s = const.tile([128, KD, 128], BF16)
            wk_s = const.tile([128, KD, 128], BF16)
            wv_s = const.tile([128, KD, 128], BF16)
            wp_s = const.tile([128, D], BF16)
            bq_s = const.tile([CPC, 1], F32)
            bk_s = const.tile([CPC, 1], F32)
            bv_s = const.tile([CPC, 1], F32)

            def setup():
                for w_d, w_s in ((wq_d, wq_s), (wk_d, wk_s), (wv_d, wv_s)):
                    nc.sync.dma_start(
                        w_s[:], w_d.ap().rearrange("(a p) m -> p a m", p=128))
                nc.sync.dma_start(wp_s[:], wp_d.ap()[:, :])
                nc.sync.dma_start(bq_s[:], bq_d.ap()[:, :])
                nc.sync.dma_start(bk_s[:], bk_d.ap()[:, :])
                nc.sync.dma_start(bv_s[:], bv_d.ap()[:, :])
                make_identity(nc, ident[:])
                nc.vector.memset(kTp_s[:], 0.0)
                nc.vector.memset(ones_bf[:], 1.0)
                ones_cols = v_s[:].rearrange(
                    "p (t h w) -> p t h w", t=NTT, h=HPC)[:, :, :, HD:VW]
                nc.vector.tensor_copy(
                    ones_cols,
                    ones_bf[:, 0:1].to_broadcast([128, NTT, HPC, 1]))

            def load_x(rep, tb, split=False):
                xt = xin.tile([128, KD, TB], BF16, name=f"xt_{rep}_{tb}",
                              tag="xt")
                if split:
                    nc.sync.dma_start(xt[:, 0:KH, :],
                                      xT_r[:, 0:KH, tb * TB:(tb + 1) * TB])
                    nc.sync.dma_start(xt[:, KH:KD, :],
                                      xT_r[:, KH:KD, tb * TB:(tb + 1) * TB])
                else:
                    nc.sync.dma_start(xt[:], xT_r[:, :, tb * TB:(tb + 1) * TB])
                return xt

            def emit_qkv_tb(rep, tb, xt, parts=("q", "k", "v")):
                """qkv projections + padded-kT / transposed-v writes for one
                512-token block."""
                lsl = slice(tb * TB, (tb + 1) * TB)
                sel = (("q", wq_s, bq_s), ("k", wk_s, bk_s),
                       ("v", wv_s, bv_s))
                for which, w_s, b_s in (t for t in sel if t[0] in parts):
                    ps = wy_ps.tile([128, TB], F32,
                                    name=f"ps_{which}{rep}_{tb}", tag="wy")
                    for kd in range(KD):
                        nc.tensor.matmul(ps[:], w_s[:, kd, :], xt[:, kd, :],
                                         start=(kd == 0), stop=(kd == KD - 1))
                    if which == "q":
                        nc.vector.tensor_scalar_add(qT_s[:, lsl], ps[:],
                                                    b_s[:])
                    elif which == "k":
                        ps_r = ps[:].rearrange("p (t c) -> p t c", c=KT)
                        for h in range(HPC):
                            hs = slice(h * HD, (h + 1) * HD)
                            dst = kTp_v[hs, tb * TPB:(tb + 1) * TPB, h, :]
                            nc.vector.tensor_scalar_add(
                                dst, ps_r[hs, :, :], b_s[hs, :])
                    else:
                        vt = vtmp.tile([128, TB], BF16, name=f"vt{rep}_{tb}",
                                       tag="vt")
                        nc.vector.tensor_scalar_add(vt[:], ps[:], b_s[:])
                        for j in range(TPB):
                            ti = tb * TPB + j
                            pt = wy_ps.tile([128, 128], BF16,
                                            name=f"pt{rep}_{ti}", tag="wy")
                            nc.tensor.transpose(
                                pt[:], vt[:, j * 128:(j + 1) * 128], ident[:])
                            base = ti * HPC * VW
                            dst = v_s[:, base:base + HPC * VW].rearrange(
                                "p (h w) -> p h w", h=HPC)[:, :, 0:HD]
                            src = pt[:].rearrange("p (h w) -> p h w", h=HPC)
                            nc.vector.tensor_copy(dst, src)

            def emit_proj_tt(rep, b, ao, tt):
                """output projection for one 128-token block of batch b."""
                lt = ao[:, tt * 128:(tt + 1) * 128]
                yt = yout.tile([128, D], F32, name=f"yt{rep}_{b}_{tt}",
                               tag="yt")
                for half in range(2):
                    py = wy_ps.tile([128, 512], F32,
                                    name=f"py{rep}_{b}_{tt}_{half}", tag="wy")
                    nc.tensor.matmul(py[:], lt,
                                     wp_s[:, half * 512:(half + 1) * 512],
                                     start=True, stop=True)
                    dst = yt[:, half * 512:(half + 1) * 512]
                    if act_copy_frac and (tt * 2 + half) % act_copy_frac == 0:
                        nc.scalar.activation(
                            dst, py[:], mybir.ActivationFunctionType.Copy)
                    else:
                        nc.vector.tensor_copy(dst, py[:])
                nc.sync.dma_start(
                    y_d.ap()[b * S + tt * 128: b * S + (tt + 1) * 128, :],
                    yt[:])

            def emit_attn_iter(rep, b, h, qb, ao):
                """scores + exp + attn@V + normalize for one (head, qblock).

                Scores run one group ahead of attn@V in the PE stream, so
                the PE never sits on av(g)'s exp(g) wait with sc(g+1) work
                still pending behind it in program order.
                """
                hs = slice(h * HD, (h + 1) * HD)
                qsl = slice(b * S + qb * QB, b * S + (qb + 1) * QB)
                po = o_ps.tile([VW, QB], F32, name=f"po{rep}_{b}_{h}_{qb}",
                               tag="po")

                def emit_sc(g):
                    ss = s_ps.tile([128, KG, QB], F32,
                                   name=f"ss{rep}_{b}_{h}_{qb}_{g}", tag="ss")
                    for j in range(KG):
                        kt = g * KG + j
                        ti = b * NKT + kt
                        nc.tensor.matmul(ss[:, j, :], kTp_v[:, ti, h, :],
                                         qT_s[:, qsl],
                                         start=(j % 2 == 0),
                                         stop=(j % 2 == 1 or j == KG - 1))
                    pTg = p_pool.tile([128, KG, QB], BF16,
                                      name=f"pT{rep}_{b}_{h}_{qb}_{g}",
                                      tag="pT")
                    nc.scalar.activation(pTg[:], ss[:],
                                         mybir.ActivationFunctionType.Exp,
                                         scale=SCALE)
                    return pTg

                def emit_av(g, pTg):
                    for j in range(KG):
                        kt = g * KG + j
                        vb = (b * NKT + kt) * HPC * VW + h * VW
                        nc.tensor.matmul(po[:], v_s[:, vb:vb + VW],
                                         pTg[:, j, :],
                                         start=(kt == 0),
                                         stop=(kt == NKT - 1))

                pTg = emit_sc(0)
                for g in range(NG):
                    pTg_next = emit_sc(g + 1) if g + 1 < NG else None
                    emit_av(g, pTg)
                    pTg = pTg_next
                rc = rcp_pool.tile([1, QB], F32, name=f"rc{rep}_{b}_{h}_{qb}",
                                   tag="rc")
                nc.vector.reciprocal(rc[:], po[HD:VW, :])
                rcb = rcb_pool.tile([HD, QB], F32,
                                    name=f"rcb{rep}_{b}_{h}_{qb}", tag="rcb")
                nc.gpsimd.partition_broadcast(rcb[:], rc[:])
                nc.vector.scalar_tensor_tensor(
                    ao[hs, qb * QB:(qb + 1) * QB], po[0:HD, :], 1.0, rcb[:],
                    op0=mybir.AluOpType.mult, op1=mybir.AluOpType.mult)

            ao_tiles = {}
            for rep in range(reps):
                # phase B for batch 0 (standalone; nothing to overlap with)
                xt = load_x(rep, 0, split=True)
                if rep == 0:
                    setup()
                for tb in range(TBB):
                    xt_next = load_x(rep, tb + 1) if tb + 1 < TBB else None
                    emit_qkv_tb(rep, tb, xt)
                    xt = xt_next

                for b in range(B):
                    ao = ao_pool.tile([128, S], BF16, name=f"ao{rep}_{b}",
                                      tag="ao")
                    ao_tiles[(rep, b)] = ao
                    # prefetch first x block of batch b+1
                    if b + 1 < B:
                        xt = load_x(rep, (b + 1) * TBB)
                    slots = NIT // TBB      # iterations per qkv filler block
                    NPT = (S // 128) // NIT  # proj blocks per iteration
                    for it in range(NIT):
                        h, qb = divmod(it, NQB)
                        emit_attn_iter(rep, b, h, qb, ao)
                        # filler: proj block(s) of batch b-1 per iteration
                        if b > 0:
                            for tt in range(it * NPT, (it + 1) * NPT):
                                emit_proj_tt(rep, b - 1,
                                             ao_tiles[(rep, b - 1)], tt)
                        # filler: one qkv block of batch b+1 per `slots` iters
                        if b + 1 < B and it % slots == slots - 1:
                            tb = (b + 1) * TBB + it // slots
                            xt_next = (load_x(rep, tb + 1)
                                       if it // slots + 1 < TBB else None)
                            emit_qkv_tb(rep, tb, xt)
                            xt = xt_next
                # tail: projection of the last batch
                for tt in range(S // 128):
                    emit_proj_tt(rep, B - 1, ao_tiles[(rep, B - 1)], tt)

    nc.compile()
    return nc


def shard_inputs(x, w_qkv, b_qkv, w_proj, B=4, S=2048, D=1024, H=16):
    """Host-side sharding: returns in_maps for the 8 cores."""
    HD = D // H
    HPC = H // N_CORES
    CPC = HPC * HD
    T = B * S
    x = np.asarray(x, dtype=np.float32)
    w_qkv = np.asarray(w_qkv, dtype=np.float32)
    b_qkv = np.asarray(b_qkv, dtype=np.float32)
    w_proj = np.asarray(w_proj, dtype=np.float32)
    bf = ml_dtypes.bfloat16
    xT = np.ascontiguousarray(x.reshape(T, D).T.astype(bf))
    in_maps = []
    for c in range(N_CORES):
        sl = slice(c * CPC, (c + 1) * CPC)
        in_maps.append({
            "xT": xT,
            "wq": np.ascontiguousarray(w_qkv[:, 0 * D:1 * D][:, sl]
                                       .astype(bf)),
            "wk": np.ascontiguousarray(w_qkv[:, 1 * D:2 * D][:, sl]
                                       .astype(bf)),
            "wv": np.ascontiguousarray(w_qkv[:, 2 * D:3 * D][:, sl]
                                       .astype(bf)),
            "bq": np.ascontiguousarray(b_qkv[0 * D:1 * D][sl]).reshape(CPC, 1),
            "bk": np.ascontiguousarray(b_qkv[1 * D:2 * D][sl]).reshape(CPC, 1),
            "bv": np.ascontiguousarray(b_qkv[2 * D:3 * D][sl]).reshape(CPC, 1),
            "wp": np.ascontiguousarray(w_proj[sl, :].astype(bf)),
        })
    return in_maps


_NC_CACHE = {}


def _get_nc():
    if "nc" not in _NC_CACHE:
        _NC_CACHE["nc"] = build_core_program()
    return _NC_CACHE["nc"]


def kernel(x, w_qkv, b_qkv, w_proj, b_proj, _trace=False):
    B, S, D = 4, 2048, 1024
    nc = _get_nc()
    in_maps = shard_inputs(x, w_qkv, b_qkv, w_proj, B=B, S=S, D=D)
    res = run_bass_kernel_spmd(nc, in_maps, core_ids=list(range(N_CORES)),
                               trace=_trace)
    y = res.results[0]["y"].astype(np.float64)
    for i in range(1, N_CORES):
        y += res.results[i]["y"]
    y += np.asarray(b_proj, dtype=np.float64)
    out = y.astype(np.float32).reshape(B, S, D)
    if _trace:
        return out, res
    return out


# revision 13
# speedup vs baseline: 1.1271x; 1.0293x over previous
"""Multi-head attention (B=4, S=2048, D=1024, H=16) on 8 trn2 NeuronCores.

Sharding: tensor-parallel over heads - 2 heads per core. Each core computes
qkv for its 128 channels (column-parallel), full attention for its 8
(batch, head) pairs, and a row-parallel slice of the output projection.
Host gathers the 8 partial projection outputs and sums them (+ b_proj).

All matmul operands are bf16 (PSUM accumulation stays f32). HW-measured
rates motivating the layout (per matmul instruction, warm):
  - [128,128] x [128,N] runs at ~N*0.42 ns (full rate) for N>=256
  - contraction over only 64 partitions runs at HALF rate regardless of N
So scores use a ZERO-PADDED kT: for each 128-ki tile and head h, a
[128,128] stationary block holding kT in partition rows 64h..64h+63 and
zeros in the other 64 rows. The moving operand is the plain stacked qT
(head0 dims in partitions 0-63, head1 in 64-127), so the zero rows kill
the other head's contribution and each score matmul contracts over the
full 128 partitions at full rate. No partition shifts are needed to build
the padded kT: head h's qkv bias-add writes its rows in place.

Phase C is ScalarE-bound (softmax exp); to keep the PE busy during exp
waits, the attention loop for batch b is interleaved with PE filler work:
the output projection of batch b-1 (one 128-token block per (head,qblock)
iteration) and the qkv projections of batch b+1 (one 512-token block per
four iterations). attn@V matmuls issue group-by-group right after each
exp so po accumulation overlaps scoring of the next group. v tiles carry
a ones column so the attn@V chain also emits the softmax denominator row;
normalization is a DVE reciprocal + GpSimd partition-broadcast + one
fused multiply.
"""

import numpy as np
import ml_dtypes

import concourse.bass as bass
import concourse.mybir as mybir
import concourse.tile as tile
from concourse import bacc
from concourse.bass_utils import run_bass_kernel_spmd
from concourse.masks import make_identity

F32 = mybir.dt.float32
BF16 = mybir.dt.bfloat16

N_CORES = 8


def build_core_program(B=4, S=2048, D=1024, H=16, QB=256, TB=512, reps=1,
                       bufs_x=2, bufs_pT=3, bufs_s=2, bufs_o=2, bufs_wy=2,
                       KG=4, act_copy_frac=0):
    """One core's program (SPMD: every core runs this on its own shard).

    act_copy_frac: every act_copy_frac-th projection psum->sbuf copy goes
    to ScalarE instead of DVE (0 = all on DVE).
    """
    HD = D // H                 # 64
    HPC = H // N_CORES          # heads per core = 2
    CPC = HPC * HD              # channels per core = 128
    T = B * S                   # tokens = 8192
    SCALE = float(HD) ** -0.5
    KT = 128                    # ki tile
    NKT = S // KT               # ki tiles per batch = 16
    NTT = T // KT               # token tiles total = 64
    VW = HD + 1                 # v tile width per head incl. ones col = 65
    NQB = S // QB               # q blocks per batch = 8
    KD = D // 128               # contraction tiles for qkv = 8
    KH = KD // 2
    NG = NKT // KG              # score groups per q block = 4
    TPB = TB // 128             # 128-token tiles per qkv block = 4
    TBB = S // TB               # qkv token blocks per batch = 4
    NIT = HPC * NQB             # attention iterations per batch = 16

    nc = bacc.Bacc("TRN2", target_bir_lowering=False, debug=False,
                   num_devices=N_CORES)

    xT_d = nc.dram_tensor("xT", [D, T], BF16, kind="ExternalInput")
    wq_d = nc.dram_tensor("wq", [D, CPC], BF16, kind="ExternalInput")
    wk_d = nc.dram_tensor("wk", [D, CPC], BF16, kind="ExternalInput")
    wv_d = nc.dram_tensor("wv", [D, CPC], BF16, kind="ExternalInput")
    bq_d = nc.dram_tensor("bq", [CPC, 1], F32, kind="ExternalInput")
    bk_d = nc.dram_tensor("bk", [CPC, 1], F32, kind="ExternalInput")
    bv_d = nc.dram_tensor("bv", [CPC, 1], F32, kind="ExternalInput")
    wp_d = nc.dram_tensor("wp", [CPC, D], BF16, kind="ExternalInput")
    y_d = nc.dram_tensor("y", [T, D], F32, kind="ExternalOutput")

    with tile.TileContext(nc) as tc:
        with tc.tile_pool(name="const", bufs=1) as const, \
             tc.tile_pool(name="persist", bufs=1) as persist, \
             tc.tile_pool(name="xin", bufs=bufs_x) as xin, \
             tc.tile_pool(name="vtmp", bufs=2) as vtmp, \
             tc.tile_pool(name="pT", bufs=bufs_pT) as p_pool, \
             tc.tile_pool(name="ao", bufs=2) as ao_pool, \
             tc.tile_pool(name="rcp", bufs=3) as rcp_pool, \
             tc.tile_pool(name="rcb", bufs=3) as rcb_pool, \
             tc.tile_pool(name="yout", bufs=3) as yout, \
             tc.tile_pool(name="s_ps", bufs=bufs_s, space="PSUM") as s_ps, \
             tc.tile_pool(name="o_ps", bufs=bufs_o, space="PSUM") as o_ps, \
             tc.tile_pool(name="wy_ps", bufs=bufs_wy, space="PSUM") as wy_ps:
            xT_r = xT_d.ap().rearrange("(a p) t -> p a t", p=128)
            qT_s = persist.tile([128, T], BF16)       # stacked head dims
            kTp_s = persist.tile([128, 2 * T], BF16)  # zero-padded per head
            v_s = persist.tile([128, NTT * HPC * VW], BF16)
            # padded-kT view: [128, tile, head, 128]
            kTp_v = kTp_s[:].rearrange("p (t h c) -> p t h c", h=HPC, c=KT)

            ident = const.tile([128, 128], BF16)
            ones_bf = const.tile([128, 2], BF16)
            wq_s = const.tile([128, KD, 128], BF16)
            wk_s = const.tile([128, KD, 128], BF16)
            wv_s = const.tile([128, KD, 128], BF16)
            wp_s = const.tile([128, D], BF16)
            bq_s = const.tile([CPC, 1], F32)
            bk_s = const.tile([CPC, 1], F32)
            bv_s = const.tile([CPC, 1], F32)

            def setup():
                for w_d, w_s in ((wq_d, wq_s), (wk_d, wk_s), (wv_d, wv_s)):
                    nc.sync.dma_start(
                        w_s[:], w_d.ap().rearrange("(a p) m -> p a m", p=128))
                nc.sync.dma_start(wp_s[:], wp_d.ap()[:, :])
                nc.sync.dma_start(bq_s[:], bq_d.ap()[:, :])
                nc.sync.dma_start(bk_s[:], bk_d.ap()[:, :])
                nc.sync.dma_start(bv_s[:], bv_d.ap()[:, :])
                make_identity(nc, ident[:])
                nc.vector.memset(kTp_s[:], 0.0)
                nc.vector.memset(ones_bf[:], 1.0)
                ones_cols = v_s[:].rearrange(
                    "p (t h w) -> p t h w", t=NTT, h=HPC)[:, :, :, HD:VW]
                nc.vector.tensor_copy(
                    ones_cols,
                    ones_bf[:, 0:1].to_broadcast([128, NTT, HPC, 1]))

            def load_x(rep, tb, split=False):
                xt = xin.tile([128, KD, TB], BF16, name=f"xt_{rep}_{tb}",
                              tag="xt")
                if split:
                    nc.sync.dma_start(xt[:, 0:KH, :],
                                      xT_r[:, 0:KH, tb * TB:(tb + 1) * TB])
                    nc.sync.dma_start(xt[:, KH:KD, :],
                                      xT_r[:, KH:KD, tb * TB:(tb + 1) * TB])
                else:
                    nc.sync.dma_start(xt[:], xT_r[:, :, tb * TB:(tb + 1) * TB])
                return xt

            def emit_qkv_tb(rep, tb, xt, parts=("q", "k", "v")):
                """qkv projections + padded-kT / transposed-v writes for one
                512-token block."""
                lsl = slice(tb * TB, (tb + 1) * TB)
                sel = (("q", wq_s, bq_s), ("k", wk_s, bk_s),
                       ("v", wv_s, bv_s))
                for which, w_s, b_s in (t for t in sel if t[0] in parts):
                    ps = wy_ps.tile([128, TB], F32,
                                    name=f"ps_{which}{rep}_{tb}", tag="wy")
                    for kd in range(KD):
                        nc.tensor.matmul(ps[:], w_s[:, kd, :], xt[:, kd, :],
                                         start=(kd == 0), stop=(kd == KD - 1))
                    if which == "q":
                        nc.vector.tensor_scalar_add(qT_s[:, lsl], ps[:],
                                                    b_s[:])
                    elif which == "k":
                        ps_r = ps[:].rearrange("p (t c) -> p t c", c=KT)
                        for h in range(HPC):
                            hs = slice(h * HD, (h + 1) * HD)
                            dst = kTp_v[hs, tb * TPB:(tb + 1) * TPB, h, :]
                            nc.vector.tensor_scalar_add(
                                dst, ps_r[hs, :, :], b_s[hs, :])
                    else:
                        vt = vtmp.tile([128, TB], BF16, name=f"vt{rep}_{tb}",
                                       tag="vt")
                        nc.vector.tensor_scalar_add(vt[:], ps[:], b_s[:])
                        for j in range(TPB):
                            ti = tb * TPB + j
                            pt = wy_ps.tile([128, 128], BF16,
                                            name=f"pt{rep}_{ti}", tag="wy")
                            nc.tensor.transpose(
                                pt[:], vt[:, j * 128:(j + 1) * 128], ident[:])
                            base = ti * HPC * VW
                            dst = v_s[:, base:base + HPC * VW].rearrange(
                                "p (h w) -> p h w", h=HPC)[:, :, 0:HD]
                            src = pt[:].rearrange("p (h w) -> p h w", h=HPC)
                            nc.vector.tensor_copy(dst, src)

            def emit_proj_tt(rep, b, ao, tt):
                """output projection for one 128-token block of batch b."""
                lt = ao[:, tt * 128:(tt + 1) * 128]
                yt = yout.tile([128, D], F32, name=f"yt{rep}_{b}_{tt}",
                               tag="yt")
                for half in range(2):
                    py = wy_ps.tile([128, 512], F32,
                                    name=f"py{rep}_{b}_{tt}_{half}", tag="wy")
                    nc.tensor.matmul(py[:], lt,
                                     wp_s[:, half * 512:(half + 1) * 512],
                                     start=True, stop=True)
                    dst = yt[:, half * 512:(half + 1) * 512]
                    if act_copy_frac and (tt * 2 + half) % act_copy_frac == 0:
                        nc.scalar.activation(
                            dst, py[:], mybir.ActivationFunctionType.Copy)
                    else:
                        nc.vector.tensor_copy(dst, py[:])
                nc.sync.dma_start(
                    y_d.ap()[b * S + tt * 128: b * S + (tt + 1) * 128, :],
                    yt[:])

            def emit_attn_iter(rep, b, h, qb, ao):
                """scores + exp + attn@V + normalize for one (head, qblock).

                Scores run one group ahead of attn@V in the PE stream, so
                the PE never sits on av(g)'s exp(g) wait with sc(g+1) work
                still pending behind it in program order.
                """
                hs = slice(h * HD, (h + 1) * HD)
                qsl = slice(b * S + qb * QB, b * S + (qb + 1) * QB)
                po = o_ps.tile([VW, QB], F32, name=f"po{rep}_{b}_{h}_{qb}",
                               tag="po")

                def emit_sc(g):
                    ss = s_ps.tile([128, KG, QB], F32,
                                   name=f"ss{rep}_{b}_{h}_{qb}_{g}", tag="ss")
                    for j in range(KG):
                        kt = g * KG + j
                        ti = b * NKT + kt
                        nc.tensor.matmul(ss[:, j, :], kTp_v[:, ti, h, :],
                                         qT_s[:, qsl],
                                         start=(j % 2 == 0),
                                         stop=(j % 2 == 1 or j == KG - 1))
                    pTg = p_pool.tile([128, KG, QB], BF16,
                                      name=f"pT{rep}_{b}_{h}_{qb}_{g}",
                                      tag="pT")
                    nc.scalar.activation(pTg[:], ss[:],
                                         mybir.ActivationFunctionType.Exp,
                                         scale=SCALE)
                    return pTg

                def emit_av(g, pTg):
                    for j in range(KG):
                        kt = g * KG + j
                        vb = (b * NKT + kt) * HPC * VW + h * VW
                        nc.tensor.matmul(po[:], v_s[:, vb:vb + VW],
                                         pTg[:, j, :],
                                         start=(kt == 0),
                                         stop=(kt == NKT - 1))

                pTg = emit_sc(0)
                for g in range(NG):
                    pTg_next = emit_sc(g + 1) if g + 1 < NG else None
                    emit_av(g, pTg)
                    pTg = pTg_next
                rc = rcp_pool.tile([1, QB], F32, name=f"rc{rep}_{b}_{h}_{qb}",
                                   tag="rc")
                nc.vector.reciprocal(rc[:], po[HD:VW, :])
                rcb = rcb_pool.tile([HD, QB], F32,
                                    name=f"rcb{rep}_{b}_{h}_{qb}", tag="rcb")
                nc.gpsimd.partition_broadcast(rcb[:], rc[:])
                nc.vector.scalar_tensor_tensor(
                    ao[hs, qb * QB:(qb + 1) * QB], po[0:HD, :], 1.0, rcb[:],
                    op0=mybir.AluOpType.mult, op1=mybir.AluOpType.mult)

            ao_tiles = {}
            for rep in range(reps):
                # phase B for batch 0 (standalone; nothing to overlap with)
                xt = load_x(rep, 0, split=True)
                if rep == 0:
                    setup()
                for tb in range(TBB):
                    xt_next = load_x(rep, tb + 1) if tb + 1 < TBB else None
                    emit_qkv_tb(rep, tb, xt)
                    xt = xt_next

                for b in range(B):
                    ao = ao_pool.tile([128, S], BF16, name=f"ao{rep}_{b}",
                                      tag="ao")
                    ao_tiles[(rep, b)] = ao
                    # prefetch first x block of batch b+1
                    if b + 1 < B:
                        xt = load_x(rep, (b + 1) * TBB)
                    slots = NIT // TBB      # iterations per qkv filler block
                    NPT = (S // 128) // NIT  # proj blocks per iteration
                    for it in range(NIT):
                        h, qb = divmod(it, NQB)
                        emit_attn_iter(rep, b, h, qb, ao)
                        # filler: proj block(s) of batch b-1 per iteration
                        if b > 0:
                            for tt in range(it * NPT, (it + 1) * NPT):
                                emit_proj_tt(rep, b - 1,
                                             ao_tiles[(rep, b - 1)], tt)
                        # filler: one qkv block of batch b+1 per `slots` iters
                        if b + 1 < B and it % slots == slots - 1:
                            tb = (b + 1) * TBB + it // slots
                            xt_next = (load_x(rep, tb + 1)
                                       if it // slots + 1 < TBB else None)
                            emit_qkv_tb(rep, tb, xt)
                            xt = xt_next
                # tail: projection of the last batch
                for tt in range(S // 128):
                    emit_proj_tt(rep, B - 1, ao_tiles[(rep, B - 1)], tt)

    nc.compile()
    return nc


def shard_inputs(x, w_qkv, b_qkv, w_proj, B=4, S=2048, D=1024, H=16):
    """Host-side sharding: returns in_maps for the 8 cores."""
    HD = D // H
    HPC = H // N_CORES
    CPC = HPC * HD
    T = B * S
    x = np.asarray(x, dtype=np.float32)
    w_qkv = np.asarray(w_qkv, dtype=np.float32)
    b_qkv = np.asarray(b_qkv, dtype=np.float32)
    w_proj = np.asarray(w_proj, dtype=np.float32)
    bf = ml_dtypes.bfloat16
    xT = np.ascontiguousarray(x.reshape(T, D).T.astype(bf))
    in_maps = []
    for c in range(N_CORES):
        sl = slice(c * CPC, (c + 1) * CPC)
        in_maps.append({
            "xT": xT,
            "wq": np.ascontiguousarray(w_qkv[:, 0 * D:1 * D][:, sl]
                                       .astype(bf)),
            "wk": np.ascontiguousarray(w_qkv[:, 1 * D:2 * D][:, sl]
                                       .astype(bf)),
            "wv": np.ascontiguousarray(w_qkv[:, 2 * D:3 * D][:, sl]
                                       .astype(bf)),
            "bq": np.ascontiguousarray(b_qkv[0 * D:1 * D][sl]).reshape(CPC, 1),
            "bk": np.ascontiguousarray(b_qkv[1 * D:2 * D][sl]).reshape(CPC, 1),
            "bv": np.ascontiguousarray(b_qkv[2 * D:3 * D][sl]).reshape(CPC, 1),
            "wp": np.ascontiguousarray(w_proj[sl, :].astype(bf)),
        })
    return in_maps


_NC_CACHE = {}


def _get_nc():
    if "nc" not in _NC_CACHE:
        _NC_CACHE["nc"] = build_core_program()
    return _NC_CACHE["nc"]


def kernel(x, w_qkv, b_qkv, w_proj, b_proj, _trace=False):
    B, S, D = 4, 2048, 1024
    nc = _get_nc()
    in_maps = shard_inputs(x, w_qkv, b_qkv, w_proj, B=B, S=S, D=D)
    res = run_bass_kernel_spmd(nc, in_maps, core_ids=list(range(N_CORES)),
                               trace=_trace)
    y = res.results[0]["y"].astype(np.float64)
    for i in range(1, N_CORES):
        y += res.results[i]["y"]
    y += np.asarray(b_proj, dtype=np.float64)
    out = y.astype(np.float32).reshape(B, S, D)
    if _trace:
        return out, res
    return out
